# revision 23
# baseline (speedup 1.0000x reference)
"""LoRA Linear kernel for Trainium2, 8 NeuronCores.

Computes out = x @ (W + lora_A @ lora_B)^T + bias for
x [4, 2048, 4096], W [4096, 4096], lora_A [4096, 16], lora_B [16, 4096].

Sharding: 2-way over tokens (M = 8192 -> 4096/core) x 4-way over
out_features (4096 -> 1024/core). Operands are bf16 (fp32 PSUM
accumulation); outputs are stored bf16 and upcast on host. The LoRA
delta is folded into W^T on-device with rank-16 matmuls
(Wtot^T = W^T + B^T A^T), added in place over the streamed W^T.

Schedule (v7). The startup critical path is the 8 MiB W^T stream, and
DMA efficiency is line-length-bound, so the host pre-arranges W^T in
the exact SBUF image layout [p, os, ki, o] (os-major) and it streams as
2x 8 chunks of 4 k-tiles (4 KiB lines) striped over the three
DMA-capable queues. The kernel runs in two half-streams: while the os0
half of W^T lands, three lead token-tiles accumulate their os0 pass
interleaved with the os0 fold at ki pace; ditto for os1. By the time
the leads finish, wtot is fully resident and the remaining 29 tiles run
os-outer at the PE floor with a 6-deep PSUM rotation, one combined
256 KiB store per tile (2 KiB lines). The final tile's stores are
partition-split across two queues to shorten the tail.
"""

import ml_dtypes

import numpy as np

import concourse.bass as bass
import concourse.bacc as bacc
import concourse.mybir as mybir
import concourse.tile as tile
from concourse.bass_utils import run_bass_kernel_spmd

IN_F = 4096
OUT_F = 4096
RANK = 16
BATCH, SEQ = 4, 2048
M_TOT = BATCH * SEQ          # 8192 tokens
MG, OG = 2, 4                # shard grid: token-groups x outfeature-groups
M_LOC = M_TOT // MG          # 4096 tokens per core
O_LOC = OUT_F // OG          # 1024 out features per core
P = 128
KI = IN_F // P               # 32 contraction tiles
NF = 512                     # matmul moving free dim (one PSUM bank)
OS = O_LOC // NF             # 2 output column passes
MT = M_LOC // P              # 32 token tiles per core
NLEAD = 6
LAG = 4                      # fold-steps between consecutive leads' starts
WCH = 4                      # ki tiles per W-stream chunk
NCH = KI // WCH              # chunks per os half

F32 = mybir.dt.float32
BF16 = mybir.dt.bfloat16

_cache = {}


def _build():
    nc = bacc.Bacc(None, target_bir_lowering=False)

    # x pre-tiled on host to [MT, P, KI, P]: (mt, i_within, i_tile, m)
    xt = nc.dram_tensor("xt", [MT, P, KI, P], BF16, kind="ExternalInput")
    # W^T pre-arranged on host to the exact wtot SBUF image [p, os, ki, o]
    wt = nc.dram_tensor("wt", [P, OS, KI, NF], BF16, kind="ExternalInput")
    lb = nc.dram_tensor("lb", [RANK, IN_F], BF16, kind="ExternalInput")
    at = nc.dram_tensor("at", [RANK, O_LOC], BF16, kind="ExternalInput")
    br = nc.dram_tensor("br", [P, O_LOC], BF16, kind="ExternalInput")
    out = nc.dram_tensor("out", [M_LOC, O_LOC], BF16, kind="ExternalOutput")

    with tile.TileContext(nc) as tc:
        with (
            tc.tile_pool(name="const", bufs=1) as const_pool,
            tc.tile_pool(name="xin", bufs=7) as xin_pool,
            tc.tile_pool(name="outs", bufs=8) as out_pool,
            tc.tile_pool(name="psum", bufs=2, space="PSUM") as psum_pool,
            tc.tile_pool(name="psum_mm", bufs=6, space="PSUM") as psum_mm_pool,
        ):
            # resident folded weight, [i_within, os, i_tile, o]
            wtot = const_pool.tile([P, OS, KI, NF], BF16, name="wtot")
            a_sb = const_pool.tile([RANK, O_LOC], BF16, name="a_sb")
            b_sb = const_pool.tile([RANK, IN_F], BF16, name="b_sb")
            bias_sb = const_pool.tile([P, O_LOC], BF16, name="bias_sb")

            x_tiles = {}

            def load_x(mt):
                x_tile = xin_pool.tile([P, KI, P], BF16, name="x_tile", tag="x_tile")
                eng = nc.sync if mt % 2 == 0 else nc.gpsimd
                eng.dma_start(x_tile[:], xt[mt])
                return x_tile

            def w_chunk(os_, c, eng):
                eng.dma_start(
                    wtot[:, os_, c * WCH : (c + 1) * WCH, :],
                    wt[:, os_, c * WCH : (c + 1) * WCH, :],
                )

            # ---- DMA issue order matters: per-queue FIFO (only sync /
            # scalar / gpsimd can issue DMAs). a/b land in parallel on two
            # queues, then the W^T stream and the lead x tiles interleave
            # in roughly the order the staggered leads consume them.
            nc.gpsimd.dma_start(a_sb[:], at[:])
            nc.scalar.dma_start(b_sb[:], lb[:])
            x_tiles[0] = load_x(0)
            x_tiles[1] = load_x(1)
            w_chunk(0, 0, nc.scalar)
            x_tiles[2] = load_x(2)
            w_chunk(0, 1, nc.sync)
            w_chunk(0, 2, nc.gpsimd)
            w_chunk(0, 3, nc.scalar)
            x_tiles[3] = load_x(3)
            w_chunk(0, 4, nc.sync)
            x_tiles[4] = load_x(4)
            w_chunk(0, 5, nc.gpsimd)
            w_chunk(0, 6, nc.scalar)
            x_tiles[5] = load_x(5)
            w_chunk(0, 7, nc.sync)
            w_engines = [nc.scalar, nc.sync, nc.gpsimd]
            for c in range(NCH):
                w_chunk(1, c, w_engines[c % 3])
            nc.gpsimd.dma_start(bias_sb[:], br[:])

            def new_otile():
                return out_pool.tile([P, O_LOC], BF16, name="o_tile", tag="o_tile")

            def add_bias(os_, psum, o_tile):
                nc.vector.tensor_add(
                    out=o_tile[:, os_ * NF : (os_ + 1) * NF],
                    in0=psum[:],
                    in1=bias_sb[:, os_ * NF : (os_ + 1) * NF],
                )

            # ---- lead tiles 0..NLEAD-1: one os half at a time, at ki pace
            # with the fold, each lead's accumulation staggered LAG fold
            # steps behind the previous (accumulation order over ki is
            # free) so x arrivals and W chunks are consumed in stream
            # order; the skipped head kis catch up after the fold.
            def lead_mm(mt, os_, ki, psum):
                # each (mt, os_) pass touches every ki exactly once; ki=0
                # is always issued first and ki=KI-1 last
                nc.tensor.matmul(
                    psum[:],
                    x_tiles[mt][:, ki, :],
                    wtot[:, os_, ki, :],
                    start=(ki == 0),
                    stop=(ki == KI - 1),
                )

            lead_otiles = [new_otile() for _ in range(NLEAD)]
            for os_ in range(OS):
                lead_psums = [
                    psum_mm_pool.tile([P, NF], F32, name=f"lpsum_{mt}_{os_}", tag="mm")
                    for mt in range(NLEAD)
                ]
                for s in range(KI):
                    dpsum = psum_pool.tile([P, NF], F32, name="dpsum", tag="dpsum")
                    nc.tensor.matmul(
                        dpsum[:],
                        b_sb[:, s * P : (s + 1) * P],
                        a_sb[:, os_ * NF : (os_ + 1) * NF],
                        start=True,
                        stop=True,
                    )
                    nc.vector.tensor_add(
                        out=wtot[:, os_, s, :],
                        in0=dpsum[:],
                        in1=wtot[:, os_, s, :],
                    )
                    for mt in range(NLEAD):
                        k = s - LAG * mt
                        if k >= 0:
                            lead_mm(mt, os_, k, lead_psums[mt])
                # catch-up: the head kis each staggered lead skipped
                for mt in range(1, NLEAD):
                    for k in range(KI - LAG * mt, KI):
                        lead_mm(mt, os_, k, lead_psums[mt])
                for mt in range(NLEAD):
                    add_bias(os_, lead_psums[mt], lead_otiles[mt])
            for mt in range(NLEAD):
                eng = nc.scalar if mt % 2 == 0 else nc.sync
                eng.dma_start(out[mt * P : (mt + 1) * P, :], lead_otiles[mt][:])

            # ---- m_tiles NLEAD..MT-1, os-outer so os0's bias-add overlaps
            # os1's accumulation; one combined store per tile. The final
            # tile's stores are partition-split across two queues.
            for mt in range(NLEAD, MT):
                x_tile = x_tiles.get(mt) or load_x(mt)
                o_tile = new_otile()
                for os_ in range(OS):
                    psum = psum_mm_pool.tile([P, NF], F32, name=f"psum{os_}", tag="mm")
                    for ki in range(KI):
                        nc.tensor.matmul(
                            psum[:],
                            x_tile[:, ki, :],
                            wtot[:, os_, ki, :],
                            start=(ki == 0),
                            stop=(ki == KI - 1),
                        )
                    add_bias(os_, psum, o_tile)
                if mt == MT - 1:
                    nc.scalar.dma_start(
                        out[mt * P : mt * P + 64, :], o_tile[0:64, :]
                    )
                    nc.sync.dma_start(
                        out[mt * P + 64 : (mt + 1) * P, :], o_tile[64:128, :]
                    )
                else:
                    nc.scalar.dma_start(out[mt * P : (mt + 1) * P, :], o_tile[:])
    nc.finalize()
    return nc


def kernel(x, W, bias, lora_A, lora_B):
    x = np.asarray(x, dtype=np.float32)
    W = np.asarray(W, dtype=np.float32)
    bias = np.asarray(bias, dtype=np.float32)
    lora_A = np.asarray(lora_A, dtype=np.float32)
    lora_B = np.asarray(lora_B, dtype=np.float32)

    if "nc" not in _cache:
        _cache["nc"] = _build()
    nc = _cache["nc"]

    xr = x.reshape(M_TOT, IN_F).astype(ml_dtypes.bfloat16)
    in_maps = []
    for c in range(8):
        mg, og = c % MG, c // MG
        xs = xr[mg * M_LOC : (mg + 1) * M_LOC]
        # [M_LOC, IN_F] -> (mt, m, ki, p) -> (mt, p, ki, m)
        xs = np.ascontiguousarray(xs.reshape(MT, P, KI, P).transpose(0, 3, 2, 1))
        # W^T slice [IN_F, O_LOC] -> (ki, p, os, o') -> (p, os, ki, o')
        wslice = W[og * O_LOC : (og + 1) * O_LOC].T.astype(ml_dtypes.bfloat16)
        wslice = np.ascontiguousarray(
            wslice.reshape(KI, P, OS, NF).transpose(1, 2, 0, 3)
        )
        in_maps.append(
            {
                "xt": xs,
                "wt": wslice,
                "lb": np.ascontiguousarray(lora_B.astype(ml_dtypes.bfloat16)),
                "at": np.ascontiguousarray(
                    lora_A[og * O_LOC : (og + 1) * O_LOC].T.astype(ml_dtypes.bfloat16)
                ),
                "br": np.ascontiguousarray(
                    np.broadcast_to(
                        bias[og * O_LOC : (og + 1) * O_LOC].astype(ml_dtypes.bfloat16),
                        (P, O_LOC),
                    )
                ),
            }
        )

    res = run_bass_kernel_spmd(nc, in_maps, core_ids=list(range(8)))

    out = np.empty((M_TOT, OUT_F), dtype=np.float32)
    for c in range(8):
        mg, og = c % MG, c // MG
        out[mg * M_LOC : (mg + 1) * M_LOC, og * O_LOC : (og + 1) * O_LOC] = res.results[
            c
        ]["out"].astype(np.float32)
    return out.reshape(BATCH, SEQ, OUT_F)


# revision 27
# speedup vs baseline: 1.0205x; 1.0205x over previous
"""LoRA Linear kernel for Trainium2, 8 NeuronCores.

Computes out = x @ (W + lora_A @ lora_B)^T + bias for
x [4, 2048, 4096], W [4096, 4096], lora_A [4096, 16], lora_B [16, 4096].

Sharding: 2-way over tokens (M = 8192 -> 4096/core) x 4-way over
out_features (4096 -> 1024/core). Operands are bf16 (fp32 PSUM
accumulation); outputs are stored bf16 and upcast on host. The LoRA
delta is folded into W^T on-device with rank-16 matmuls
(Wtot^T = W^T + B^T A^T), added in place over the streamed W^T.

Schedule (v7). The startup critical path is the 8 MiB W^T stream, and
DMA efficiency is line-length-bound, so the host pre-arranges W^T in
the exact SBUF image layout [p, os, ki, o] (os-major) and it streams as
2x 8 chunks of 4 k-tiles (4 KiB lines) striped over the three
DMA-capable queues. The kernel runs in two half-streams: while the os0
half of W^T lands, three lead token-tiles accumulate their os0 pass
interleaved with the os0 fold at ki pace; ditto for os1. By the time
the leads finish, wtot is fully resident and the remaining 29 tiles run
os-outer at the PE floor with a 6-deep PSUM rotation, one combined
256 KiB store per tile (2 KiB lines). The final tile's stores are
partition-split across two queues to shorten the tail.
"""

import ml_dtypes

import numpy as np

import concourse.bass as bass
import concourse.bacc as bacc
import concourse.mybir as mybir
import concourse.tile as tile
from concourse.bass_utils import run_bass_kernel_spmd

IN_F = 4096
OUT_F = 4096
RANK = 16
BATCH, SEQ = 4, 2048
M_TOT = BATCH * SEQ          # 8192 tokens
MG, OG = 2, 4                # shard grid: token-groups x outfeature-groups
M_LOC = M_TOT // MG          # 4096 tokens per core
O_LOC = OUT_F // OG          # 1024 out features per core
P = 128
KI = IN_F // P               # 32 contraction tiles
NF = 512                     # matmul moving free dim (one PSUM bank)
OS = O_LOC // NF             # 2 output column passes
MT = M_LOC // P              # 32 token tiles per core
NLEAD = 3
WCH = 4                      # ki tiles per W-stream chunk
NCH = KI // WCH              # chunks per os half

F32 = mybir.dt.float32
BF16 = mybir.dt.bfloat16

_cache = {}


def _build():
    nc = bacc.Bacc(None, target_bir_lowering=False)

    # x pre-tiled on host to [MT, P, KI, P]: (mt, i_within, i_tile, m)
    xt = nc.dram_tensor("xt", [MT, P, KI, P], BF16, kind="ExternalInput")
    # W^T pre-arranged on host to the exact wtot SBUF image [p, os, ki, o]
    wt = nc.dram_tensor("wt", [P, OS, KI, NF], BF16, kind="ExternalInput")
    lb = nc.dram_tensor("lb", [RANK, IN_F], BF16, kind="ExternalInput")
    at = nc.dram_tensor("at", [RANK, O_LOC], BF16, kind="ExternalInput")
    br = nc.dram_tensor("br", [P, O_LOC], BF16, kind="ExternalInput")
    out = nc.dram_tensor("out", [M_LOC, O_LOC], BF16, kind="ExternalOutput")

    with tile.TileContext(nc) as tc:
        with (
            tc.tile_pool(name="const", bufs=1) as const_pool,
            tc.tile_pool(name="xin", bufs=5) as xin_pool,
            tc.tile_pool(name="outs", bufs=6) as out_pool,
            tc.tile_pool(name="psum", bufs=2, space="PSUM") as psum_pool,
            tc.tile_pool(name="psum_mm", bufs=6, space="PSUM") as psum_mm_pool,
        ):
            # resident folded weight, [i_within, os, i_tile, o]
            wtot = const_pool.tile([P, OS, KI, NF], BF16, name="wtot")
            a_sb = const_pool.tile([RANK, O_LOC], BF16, name="a_sb")
            b_sb = const_pool.tile([RANK, IN_F], BF16, name="b_sb")
            bias_sb = const_pool.tile([P, O_LOC], BF16, name="bias_sb")

            x_tiles = {}

            def load_x(mt):
                x_tile = xin_pool.tile([P, KI, P], BF16, name="x_tile", tag="x_tile")
                eng = nc.sync if mt % 2 == 0 else nc.gpsimd
                eng.dma_start(x_tile[:], xt[mt])
                return x_tile

            def w_chunk(os_, c, eng):
                eng.dma_start(
                    wtot[:, os_, c * WCH : (c + 1) * WCH, :],
                    wt[:, os_, c * WCH : (c + 1) * WCH, :],
                )

            # ---- DMA issue order matters: per-queue FIFO (only sync /
            # scalar / gpsimd can issue DMAs). a/b land in parallel on two
            # queues, lead x tiles next, then the os0 half of W^T, the
            # next x tiles, the os1 half, then steady-state x.
            nc.gpsimd.dma_start(a_sb[:], at[:])
            nc.scalar.dma_start(b_sb[:], lb[:])
            x_tiles[0] = load_x(0)
            x_tiles[1] = load_x(1)
            x_tiles[2] = load_x(2)
            w_engines = [nc.scalar, nc.sync, nc.gpsimd]
            for c in range(NCH):
                w_chunk(0, c, w_engines[c % 3])
            x_tiles[3] = load_x(3)
            nc.gpsimd.dma_start(bias_sb[:], br[:])
            for c in range(NCH):
                w_chunk(1, c, w_engines[c % 3])
            x_tiles[4] = load_x(4)

            def new_otile():
                return out_pool.tile([P, O_LOC], BF16, name="o_tile", tag="o_tile")

            def add_bias(os_, psum, o_tile):
                nc.vector.tensor_add(
                    out=o_tile[:, os_ * NF : (os_ + 1) * NF],
                    in0=psum[:],
                    in1=bias_sb[:, os_ * NF : (os_ + 1) * NF],
                )

            # ---- lead tiles 0..NLEAD-1: one os half at a time, at ki pace
            # with the fold, so each half-stream's latency is covered.
            lead_otiles = [new_otile() for _ in range(NLEAD)]
            for os_ in range(OS):
                lead_psums = [
                    psum_mm_pool.tile([P, NF], F32, name=f"lpsum_{mt}_{os_}", tag="mm")
                    for mt in range(NLEAD)
                ]
                for ki in range(KI):
                    dpsum = psum_pool.tile([P, NF], F32, name="dpsum", tag="dpsum")
                    nc.tensor.matmul(
                        dpsum[:],
                        b_sb[:, ki * P : (ki + 1) * P],
                        a_sb[:, os_ * NF : (os_ + 1) * NF],
                        start=True,
                        stop=True,
                    )
                    nc.vector.tensor_add(
                        out=wtot[:, os_, ki, :],
                        in0=dpsum[:],
                        in1=wtot[:, os_, ki, :],
                    )
                    for mt in range(NLEAD):
                        nc.tensor.matmul(
                            lead_psums[mt][:],
                            x_tiles[mt][:, ki, :],
                            wtot[:, os_, ki, :],
                            start=(ki == 0),
                            stop=(ki == KI - 1),
                        )
                for mt in range(NLEAD):
                    add_bias(os_, lead_psums[mt], lead_otiles[mt])
            for mt in range(NLEAD):
                nc.scalar.dma_start(out[mt * P : (mt + 1) * P, :], lead_otiles[mt][:])

            # ---- m_tiles NLEAD..MT-1, os-outer so os0's bias-add overlaps
            # os1's accumulation; one combined store per tile. The final
            # tile's stores are partition-split across two queues.
            for mt in range(NLEAD, MT):
                x_tile = x_tiles.get(mt) or load_x(mt)
                o_tile = new_otile()
                for os_ in range(OS):
                    psum = psum_mm_pool.tile([P, NF], F32, name=f"psum{os_}", tag="mm")
                    for ki in range(KI):
                        nc.tensor.matmul(
                            psum[:],
                            x_tile[:, ki, :],
                            wtot[:, os_, ki, :],
                            start=(ki == 0),
                            stop=(ki == KI - 1),
                        )
                    add_bias(os_, psum, o_tile)
                if mt == MT - 1:
                    nc.scalar.dma_start(
                        out[mt * P : mt * P + 64, :], o_tile[0:64, :]
                    )
                    nc.sync.dma_start(
                        out[mt * P + 64 : (mt + 1) * P, :], o_tile[64:128, :]
                    )
                else:
                    nc.scalar.dma_start(out[mt * P : (mt + 1) * P, :], o_tile[:])
    nc.finalize()
    return nc


def kernel(x, W, bias, lora_A, lora_B):
    x = np.asarray(x, dtype=np.float32)
    W = np.asarray(W, dtype=np.float32)
    bias = np.asarray(bias, dtype=np.float32)
    lora_A = np.asarray(lora_A, dtype=np.float32)
    lora_B = np.asarray(lora_B, dtype=np.float32)

    if "nc" not in _cache:
        _cache["nc"] = _build()
    nc = _cache["nc"]

    xr = x.reshape(M_TOT, IN_F).astype(ml_dtypes.bfloat16)
    in_maps = []
    for c in range(8):
        mg, og = c % MG, c // MG
        xs = xr[mg * M_LOC : (mg + 1) * M_LOC]
        # [M_LOC, IN_F] -> (mt, m, ki, p) -> (mt, p, ki, m)
        xs = np.ascontiguousarray(xs.reshape(MT, P, KI, P).transpose(0, 3, 2, 1))
        # W^T slice [IN_F, O_LOC] -> (ki, p, os, o') -> (p, os, ki, o')
        wslice = W[og * O_LOC : (og + 1) * O_LOC].T.astype(ml_dtypes.bfloat16)
        wslice = np.ascontiguousarray(
            wslice.reshape(KI, P, OS, NF).transpose(1, 2, 0, 3)
        )
        in_maps.append(
            {
                "xt": xs,
                "wt": wslice,
                "lb": np.ascontiguousarray(lora_B.astype(ml_dtypes.bfloat16)),
                "at": np.ascontiguousarray(
                    lora_A[og * O_LOC : (og + 1) * O_LOC].T.astype(ml_dtypes.bfloat16)
                ),
                "br": np.ascontiguousarray(
                    np.broadcast_to(
                        bias[og * O_LOC : (og + 1) * O_LOC].astype(ml_dtypes.bfloat16),
                        (P, O_LOC),
                    )
                ),
            }
        )

    res = run_bass_kernel_spmd(nc, in_maps, core_ids=list(range(8)))

    out = np.empty((M_TOT, OUT_F), dtype=np.float32)
    for c in range(8):
        mg, og = c % MG, c // MG
        out[mg * M_LOC : (mg + 1) * M_LOC, og * O_LOC : (og + 1) * O_LOC] = res.results[
            c
        ]["out"].astype(np.float32)
    return out.reshape(BATCH, SEQ, OUT_F)


# revision 28
# speedup vs baseline: 1.0308x; 1.0100x over previous
"""LoRA Linear kernel for Trainium2, 8 NeuronCores.

Computes out = x @ (W + lora_A @ lora_B)^T + bias for
x [4, 2048, 4096], W [4096, 4096], lora_A [4096, 16], lora_B [16, 4096].

Sharding: 2-way over tokens (M = 8192 -> 4096/core) x 4-way over
out_features (4096 -> 1024/core). Operands are bf16 (fp32 PSUM
accumulation); outputs are stored bf16 and upcast on host. The LoRA
delta is folded into W^T on-device with rank-16 matmuls
(Wtot^T = W^T + B^T A^T), added in place over the streamed W^T.

Schedule (v7). The startup critical path is the 8 MiB W^T stream, and
DMA efficiency is line-length-bound, so the host pre-arranges W^T in
the exact SBUF image layout [p, os, ki, o] (os-major) and it streams as
2x 8 chunks of 4 k-tiles (4 KiB lines) striped over the three
DMA-capable queues. The kernel runs in two half-streams: while the os0
half of W^T lands, three lead token-tiles accumulate their os0 pass
interleaved with the os0 fold at ki pace; ditto for os1. By the time
the leads finish, wtot is fully resident and the remaining 29 tiles run
os-outer at the PE floor with a 6-deep PSUM rotation, one combined
256 KiB store per tile (2 KiB lines). The final tile's stores are
partition-split across two queues to shorten the tail.
"""

import ml_dtypes

import numpy as np

import concourse.bass as bass
import concourse.bacc as bacc
import concourse.mybir as mybir
import concourse.tile as tile
from concourse.bass_utils import run_bass_kernel_spmd

IN_F = 4096
OUT_F = 4096
RANK = 16
BATCH, SEQ = 4, 2048
M_TOT = BATCH * SEQ          # 8192 tokens
MG, OG = 2, 4                # shard grid: token-groups x outfeature-groups
M_LOC = M_TOT // MG          # 4096 tokens per core
O_LOC = OUT_F // OG          # 1024 out features per core
P = 128
KI = IN_F // P               # 32 contraction tiles
NF = 512                     # matmul moving free dim (one PSUM bank)
OS = O_LOC // NF             # 2 output column passes
MT = M_LOC // P              # 32 token tiles per core
NLEAD = 3
WCH = 8                      # ki tiles per W-stream chunk
NCH = KI // WCH              # chunks per os half

F32 = mybir.dt.float32
BF16 = mybir.dt.bfloat16

_cache = {}


def _build():
    nc = bacc.Bacc(None, target_bir_lowering=False)

    # x pre-tiled on host to [MT, P, KI, P]: (mt, i_within, i_tile, m)
    xt = nc.dram_tensor("xt", [MT, P, KI, P], BF16, kind="ExternalInput")
    # W^T pre-arranged on host to the exact wtot SBUF image [p, os, ki, o]
    wt = nc.dram_tensor("wt", [P, OS, KI, NF], BF16, kind="ExternalInput")
    lb = nc.dram_tensor("lb", [RANK, IN_F], BF16, kind="ExternalInput")
    at = nc.dram_tensor("at", [RANK, O_LOC], BF16, kind="ExternalInput")
    br = nc.dram_tensor("br", [P, O_LOC], BF16, kind="ExternalInput")
    out = nc.dram_tensor("out", [M_LOC, O_LOC], BF16, kind="ExternalOutput")

    with tile.TileContext(nc) as tc:
        with (
            tc.tile_pool(name="const", bufs=1) as const_pool,
            tc.tile_pool(name="xin", bufs=5) as xin_pool,
            tc.tile_pool(name="outs", bufs=6) as out_pool,
            tc.tile_pool(name="psum", bufs=2, space="PSUM") as psum_pool,
            tc.tile_pool(name="psum_mm", bufs=6, space="PSUM") as psum_mm_pool,
        ):
            # resident folded weight, [i_within, os, i_tile, o]
            wtot = const_pool.tile([P, OS, KI, NF], BF16, name="wtot")
            a_sb = const_pool.tile([RANK, O_LOC], BF16, name="a_sb")
            b_sb = const_pool.tile([RANK, IN_F], BF16, name="b_sb")
            bias_sb = const_pool.tile([P, O_LOC], BF16, name="bias_sb")

            x_tiles = {}

            def load_x(mt):
                x_tile = xin_pool.tile([P, KI, P], BF16, name="x_tile", tag="x_tile")
                eng = nc.sync if mt % 2 == 0 else nc.gpsimd
                eng.dma_start(x_tile[:], xt[mt])
                return x_tile

            def w_chunk(os_, c, eng):
                eng.dma_start(
                    wtot[:, os_, c * WCH : (c + 1) * WCH, :],
                    wt[:, os_, c * WCH : (c + 1) * WCH, :],
                )

            # ---- DMA issue order matters: per-queue FIFO (only sync /
            # scalar / gpsimd can issue DMAs). a/b land in parallel on two
            # queues, lead x tiles next, then the os0 half of W^T, the
            # next x tiles, the os1 half, then steady-state x.
            nc.gpsimd.dma_start(a_sb[:], at[:])
            nc.scalar.dma_start(b_sb[:], lb[:])
            x_tiles[0] = load_x(0)
            x_tiles[1] = load_x(1)
            x_tiles[2] = load_x(2)
            w_engines = [nc.scalar, nc.sync, nc.gpsimd]
            for c in range(NCH):
                w_chunk(0, c, w_engines[c % 3])
            x_tiles[3] = load_x(3)
            nc.gpsimd.dma_start(bias_sb[:], br[:])
            for c in range(NCH):
                w_chunk(1, c, w_engines[c % 3])
            x_tiles[4] = load_x(4)

            def new_otile():
                return out_pool.tile([P, O_LOC], BF16, name="o_tile", tag="o_tile")

            def add_bias(os_, psum, o_tile):
                nc.vector.tensor_add(
                    out=o_tile[:, os_ * NF : (os_ + 1) * NF],
                    in0=psum[:],
                    in1=bias_sb[:, os_ * NF : (os_ + 1) * NF],
                )

            # ---- lead tiles 0..NLEAD-1: one os half at a time, at ki pace
            # with the fold, so each half-stream's latency is covered.
            lead_otiles = [new_otile() for _ in range(NLEAD)]
            for os_ in range(OS):
                lead_psums = [
                    psum_mm_pool.tile([P, NF], F32, name=f"lpsum_{mt}_{os_}", tag="mm")
                    for mt in range(NLEAD)
                ]
                for ki in range(KI):
                    dpsum = psum_pool.tile([P, NF], F32, name="dpsum", tag="dpsum")
                    nc.tensor.matmul(
                        dpsum[:],
                        b_sb[:, ki * P : (ki + 1) * P],
                        a_sb[:, os_ * NF : (os_ + 1) * NF],
                        start=True,
                        stop=True,
                    )
                    nc.vector.tensor_add(
                        out=wtot[:, os_, ki, :],
                        in0=dpsum[:],
                        in1=wtot[:, os_, ki, :],
                    )
                    for mt in range(NLEAD):
                        nc.tensor.matmul(
                            lead_psums[mt][:],
                            x_tiles[mt][:, ki, :],
                            wtot[:, os_, ki, :],
                            start=(ki == 0),
                            stop=(ki == KI - 1),
                        )
                for mt in range(NLEAD):
                    add_bias(os_, lead_psums[mt], lead_otiles[mt])
            for mt in range(NLEAD):
                nc.scalar.dma_start(out[mt * P : (mt + 1) * P, :], lead_otiles[mt][:])

            # ---- m_tiles NLEAD..MT-1, os-outer so os0's bias-add overlaps
            # os1's accumulation; one combined store per tile. The final
            # tile's stores are partition-split across two queues.
            for mt in range(NLEAD, MT):
                x_tile = x_tiles.get(mt) or load_x(mt)
                o_tile = new_otile()
                for os_ in range(OS):
                    psum = psum_mm_pool.tile([P, NF], F32, name=f"psum{os_}", tag="mm")
                    for ki in range(KI):
                        nc.tensor.matmul(
                            psum[:],
                            x_tile[:, ki, :],
                            wtot[:, os_, ki, :],
                            start=(ki == 0),
                            stop=(ki == KI - 1),
                        )
                    add_bias(os_, psum, o_tile)
                if mt == MT - 1:
                    nc.scalar.dma_start(
                        out[mt * P : mt * P + 64, :], o_tile[0:64, :]
                    )
                    nc.sync.dma_start(
                        out[mt * P + 64 : (mt + 1) * P, :], o_tile[64:128, :]
                    )
                else:
                    nc.scalar.dma_start(out[mt * P : (mt + 1) * P, :], o_tile[:])
    nc.finalize()
    return nc


def kernel(x, W, bias, lora_A, lora_B):
    x = np.asarray(x, dtype=np.float32)
    W = np.asarray(W, dtype=np.float32)
    bias = np.asarray(bias, dtype=np.float32)
    lora_A = np.asarray(lora_A, dtype=np.float32)
    lora_B = np.asarray(lora_B, dtype=np.float32)

    if "nc" not in _cache:
        _cache["nc"] = _build()
    nc = _cache["nc"]

    xr = x.reshape(M_TOT, IN_F).astype(ml_dtypes.bfloat16)
    in_maps = []
    for c in range(8):
        mg, og = c % MG, c // MG
        xs = xr[mg * M_LOC : (mg + 1) * M_LOC]
        # [M_LOC, IN_F] -> (mt, m, ki, p) -> (mt, p, ki, m)
        xs = np.ascontiguousarray(xs.reshape(MT, P, KI, P).transpose(0, 3, 2, 1))
        # W^T slice [IN_F, O_LOC] -> (ki, p, os, o') -> (p, os, ki, o')
        wslice = W[og * O_LOC : (og + 1) * O_LOC].T.astype(ml_dtypes.bfloat16)
        wslice = np.ascontiguousarray(
            wslice.reshape(KI, P, OS, NF).transpose(1, 2, 0, 3)
        )
        in_maps.append(
            {
                "xt": xs,
                "wt": wslice,
                "lb": np.ascontiguousarray(lora_B.astype(ml_dtypes.bfloat16)),
                "at": np.ascontiguousarray(
                    lora_A[og * O_LOC : (og + 1) * O_LOC].T.astype(ml_dtypes.bfloat16)
                ),
                "br": np.ascontiguousarray(
                    np.broadcast_to(
                        bias[og * O_LOC : (og + 1) * O_LOC].astype(ml_dtypes.bfloat16),
                        (P, O_LOC),
                    )
                ),
            }
        )

    res = run_bass_kernel_spmd(nc, in_maps, core_ids=list(range(8)))

    out = np.empty((M_TOT, OUT_F), dtype=np.float32)
    for c in range(8):
        mg, og = c % MG, c // MG
        out[mg * M_LOC : (mg + 1) * M_LOC, og * O_LOC : (og + 1) * O_LOC] = res.results[
            c
        ]["out"].astype(np.float32)
    return out.reshape(BATCH, SEQ, OUT_F)
